# revision 2
# baseline (speedup 1.0000x reference)
"""Trainium2 Bass kernel for the CustomGNNLayer problem (v4).

Wall-clock-oriented redesign. The graded metric is dominated by host prep +
axon transfer + per-call dispatch, so v2:
  - moves the tiny query-side math to host numpy (rel softmax via Wc, probs
    gather, gq = tanh(hs@Wq+b), c_q, and the final tanh(pooled@Wg+b) —
    ~0.2 GFLOP total), so Wc/Wq/Wg/onehot are never shipped;
  - ships compacted nodes as bf16 (halves bytes; PE matmuls run bf16);
  - device computes only the heavy per-slot part: X = tanh(Wn^T n + bn),
    dots = gq.X, group+global softmax, weighted sum -> pooled [E, QPC].

Device layout per core (one SPMD program, shapes uniform across cores):
  blocks b = qi*K + k; groups sorted by padded length desc; per-block class
  profile = position-wise max across cores of sorted padded lengths.
"""
import sys

sys.path.insert(0, "/opt/trn_rl_repo")

import numpy as np
from ml_dtypes import bfloat16, float8_e4m3

try:
    import jax
    jax.config.update("jax_compilation_cache_dir", "/tmp/jax_bass_cache")
    jax.config.update("jax_persistent_cache_min_compile_time_secs", 0.0)
    jax.config.update("jax_persistent_cache_min_entry_size_bytes", 0)
except Exception:
    pass

import concourse.bacc as bacc
import concourse.bass as bass
import concourse.tile as tile
from concourse import mybir
from concourse.bass_utils import run_bass_kernel_spmd

F32 = mybir.dt.float32
BF = mybir.dt.bfloat16
FP8 = mybir.dt.float8e4
AF = mybir.ActivationFunctionType
ALU = mybir.AluOpType
AX = mybir.AxisListType

Q, K, N, M = 64, 2, 32, 64
E, D, R, S = 256, 1024, 200, 8192
NCORES = 8
QPC = Q // NCORES          # 8 queries per core
NB = QPC * K               # 16 blocks per core, b = qi*K + k
PAD = 4
CHUNK = 512
ET = E // 128              # 2 e-tiles
DT = D // 128              # 8 d-tiles
INV = 1.0 / (N * M * K)


def _chunks(s):
    n = (s + CHUNK - 1) // CHUNK
    h = s // 2
    base, rem = divmod(h, n)
    sizes = [2 * (base + (1 if i < rem else 0)) for i in range(n)]
    out, off = [], 0
    for sz in sizes:
        out.append((off, sz))
        off += sz
    return out


def _host_prep(inputs):
    hs = np.ascontiguousarray(inputs["hidden_states"], dtype=np.float32)
    nodes = np.asarray(inputs["nodes"], dtype=np.float32)
    prob_idx = np.asarray(inputs["prob_idx"])
    gnn_idx = np.asarray(inputs["gnn_idx"]).astype(np.int64)
    rel_idx = np.asarray(inputs["rel_idx"]).astype(np.int64)
    Wc = np.asarray(inputs["Wc"], np.float32)
    bc = np.asarray(inputs["bc"], np.float32)
    Wq = np.asarray(inputs["Wq"], np.float32)
    bq = np.asarray(inputs["bq"], np.float32)
    Wn = np.asarray(inputs["Wn"], np.float32)
    bn = np.asarray(inputs["bn"], np.float32)

    # ---- query-side math on host (tiny) ----
    rl = hs[rel_idx] @ Wc + bc                       # [Q, R]
    rl -= rl.max(axis=1, keepdims=True)
    np.exp(rl, out=rl)
    rl /= rl.sum(axis=1, keepdims=True)
    probs10 = np.take_along_axis(
        rl, prob_idx.reshape(Q, K * N), axis=1).reshape(Q, K, N) * 10.0
    gq = np.tanh(hs[gnn_idx] @ Wq + bq)              # [Q, D]
    c_q = gq @ np.tanh(bn)                           # [Q]

    # ---- compaction profile (kept slots are the prefix of each group) ----
    mask0 = nodes[..., 0] != 0.0                     # [Q,K,N,M]
    lens = mask0.sum(axis=3, dtype=np.int64)         # [Q,K,N]
    Lg = np.minimum(((np.maximum(lens, 1) + PAD - 1) // PAD) * PAD, M)
    Lg5 = Lg.reshape(NCORES, QPC, K, N)
    order = np.argsort(-Lg5, axis=3, kind="stable")  # [C,QPC,K,N]
    Lsort = -np.sort(-Lg5, axis=3)
    prof2 = Lsort.max(axis=0).reshape(NB, N)         # [NB, N] desc, same all cores
    S_b = [int(s) for s in prof2.sum(axis=1)]
    starts = np.concatenate(
        [np.zeros((NB, 1), np.int64), np.cumsum(prof2, axis=1)], axis=1)

    segs = []       # [NB] list of (L, row0, cnt, slot_off)
    for b in range(NB):
        p = prof2[b]
        s, off, i = [], 0, 0
        while i < N:
            j = i
            while j < N and p[j] == p[i]:
                j += 1
            L = int(p[i])
            s.append((L, i, j - i, off))
            off += L * (j - i)
            i = j
        segs.append(s)

    # position -> (rank, offset) maps per block
    pos_rank, pos_off = [], []
    for b in range(NB):
        pr = np.repeat(np.arange(N), prof2[b])
        pos_rank.append(pr)
        pos_off.append(np.arange(S_b[b]) - np.repeat(starts[b, :N], prof2[b]))

    # per-core small tensors
    lens5 = lens.reshape(NCORES, QPC, K, N)
    lens_ord = np.take_along_axis(lens5, order, axis=3)       # [C,QPC,K,N]
    maskf = ((np.arange(M)[None, None, None, None, :] <
              lens_ord[..., None]).astype(np.float32) * INV)  # [C,QPC,K,N,M]
    maskf = np.ascontiguousarray(maskf.reshape(NCORES, NB, N, M))
    pr_ord = np.take_along_axis(
        probs10.reshape(NCORES, QPC, K, N), order, axis=3).reshape(NCORES, NB, N)
    prn = np.ascontiguousarray(pr_ord.transpose(0, 2, 1), np.float32)  # [C,N,NB]

    Wn_bf = Wn.astype(bfloat16)
    S_tot = sum(S_b)

    def _core_stream(c):
        qs = np.arange(c * QPC, (c + 1) * QPC)
        buf = np.empty(E * S_tot, float8_e4m3)
        off = 0
        for qi in range(QPC):
            for k in range(K):
                b = qi * K + k
                sb = S_b[b]
                idx = order[c, qi, k][pos_rank[b]] * M + pos_off[b]
                arr = nodes[qs[qi], k].reshape(N * M, E)[idx]   # [sb, E] f32
                buf[off : off + E * sb] = arr.T.astype(float8_e4m3).ravel()
                off += E * sb
        return buf

    from concurrent.futures import ThreadPoolExecutor
    with ThreadPoolExecutor(NCORES) as tpe:
        bufs = list(tpe.map(_core_stream, range(NCORES)))

    per_core = []
    for c in range(NCORES):
        qs = np.arange(c * QPC, (c + 1) * QPC)
        buf = bufs[c]
        f32b = np.concatenate([
            np.ascontiguousarray(bn.reshape(DT, 128).T).ravel(),     # [128,DT]
            maskf[c].ravel(),                                        # [NB,N,M]
            prn[c].ravel(),                                          # [N,NB]
            c_q[qs].astype(np.float32).ravel(),                      # [QPC]
        ])
        bf16b = np.concatenate([
            np.ascontiguousarray(gq[qs].T).astype(bfloat16).ravel(), # [D,QPC]
            Wn_bf.ravel(),                                           # [E,D]
            np.ones(128, bfloat16),
        ])
        per_core.append({"nodesT": buf, "f32b": f32b, "bf16b": bf16b})
    meta = {
        "gnn_idx": gnn_idx, "hs": hs,
        "Wg": np.asarray(inputs["Wg"], np.float32),
        "bg": np.asarray(inputs["bg"], np.float32),
    }
    return per_core, prof2, S_b, segs, meta


def _build_program(S_b, segs):
    nc = bacc.Bacc("TRN2", target_bir_lowering=False, debug=False,
                   num_devices=NCORES)
    S_MAX = max(S_b)
    S_tot = sum(S_b)

    O_BN, O_MF = 0, 128 * DT
    O_PRN = O_MF + NB * N * M
    O_CQ = O_PRN + N * NB
    NF32 = O_CQ + QPC
    O_WN = D * QPC
    O_ONE = O_WN + E * D
    NBF = O_ONE + 128
    d_nodesT = nc.dram_tensor("nodesT", [E * S_tot], FP8, kind="ExternalInput").ap()
    d_f32b = nc.dram_tensor("f32b", [NF32], F32, kind="ExternalInput").ap()
    d_bf16b = nc.dram_tensor("bf16b", [NBF], BF, kind="ExternalInput").ap()
    d_pooled = nc.dram_tensor("pooled", [128, ET, QPC], F32,
                              kind="ExternalOutput").ap()

    # DRAM scratch
    d_dots = nc.dram_tensor("sc_dots", [NB, N * M], F32).ap()
    d_wa = nc.dram_tensor("sc_wa", [NB, N * M], BF).ap()
    d_ginv = nc.dram_tensor("sc_ginv", [NB, 1], F32).ap()

    with tile.TileContext(nc) as tc:
        with tc.tile_pool(name="wts", bufs=1) as wts, \
             tc.tile_pool(name="big", bufs=2) as big, \
             tc.tile_pool(name="strm", bufs=4) as strm, \
             tc.tile_pool(name="sml", bufs=4) as sml, \
             tc.tile_pool(name="ps", bufs=2, space="PSUM") as ps, \
             tc.tile_pool(name="psd", bufs=2, space="PSUM") as psd, \
             tc.tile_pool(name="psw", bufs=2, space="PSUM") as psw:

            # ---------------- load constants ----------------
            sWn = wts.tile([128, ET, D], BF)
            nc.sync.dma_start(sWn, bass.AP(tensor=d_bf16b.tensor, offset=O_WN,
                                           ap=[[D, 128], [128 * D, ET], [1, D]]))
            t_gqT = wts.tile([128, DT, QPC], BF)
            nc.sync.dma_start(t_gqT, bass.AP(tensor=d_bf16b.tensor, offset=0,
                                             ap=[[QPC, 128], [128 * QPC, DT],
                                                 [1, QPC]]))
            sbn = wts.tile([128, DT], F32)
            nc.sync.dma_start(sbn, bass.AP(tensor=d_f32b.tensor, offset=O_BN,
                                           ap=[[DT, 128], [1, DT]]))
            smaskf = wts.tile([N, NB, M], F32)
            nc.sync.dma_start(smaskf, bass.AP(tensor=d_f32b.tensor, offset=O_MF,
                                              ap=[[M, N], [N * M, NB], [1, M]]))
            s_prn = wts.tile([N, NB], F32)
            nc.sync.dma_start(s_prn, bass.AP(tensor=d_f32b.tensor, offset=O_PRN,
                                             ap=[[NB, N], [1, NB]]))
            s_onesb = wts.tile([1, 128], BF)
            nc.sync.dma_start(s_onesb, bass.AP(tensor=d_bf16b.tensor, offset=O_ONE,
                                               ap=[[128, 1], [1, 128]]))
            t_pooled = wts.tile([128, ET, QPC], F32)

            # ---------------- main loop ----------------
            nt_off = 0
            from collections import defaultdict
            partials = defaultdict(list)
            for b in range(NB):
                qi, k = b // K, b % K
                sb = S_b[b]
                chs = _chunks(sb)

                t_nt8 = big.tile([128, ET, S_MAX], FP8, tag="nt8")
                nc.sync.dma_start(
                    t_nt8[:, :, 0:sb],
                    bass.AP(tensor=d_nodesT.tensor, offset=nt_off,
                            ap=[[sb, 128], [128 * sb, ET], [1, sb]]))
                nt_off += E * sb
                t_nt = big.tile([128, ET, S_MAX], BF, tag="nt")
                for et in range(ET):
                    nc.vector.tensor_copy(t_nt[:, et, 0:sb], t_nt8[:, et, 0:sb])

                # dots over the compacted stream
                t_dots = big.tile([1, S_MAX], F32, tag="dots")
                for (c0, cw) in chs:
                    p_dot = psd.tile([1, CHUNK], F32, tag="dot")
                    for dt_i in range(DT):
                        p_x = ps.tile([128, CHUNK], F32, tag="mm")
                        for et in range(ET):
                            nc.tensor.matmul(
                                p_x[:, 0:cw],
                                sWn[:, et, dt_i * 128:(dt_i + 1) * 128],
                                t_nt[:, et, c0 : c0 + cw],
                                start=(et == 0), stop=(et == ET - 1))
                        t_tx = strm.tile([128, CHUNK], BF, tag="tx")
                        nc.scalar.activation(t_tx[:, 0:cw], p_x[:, 0:cw],
                                             AF.Tanh, bias=sbn[:, dt_i : dt_i + 1],
                                             scale=1.0)
                        nc.tensor.matmul(p_dot[0:1, 0:cw],
                                         t_gqT[:, dt_i, qi : qi + 1],
                                         t_tx[:, 0:cw], start=(dt_i == 0),
                                         stop=(dt_i == DT - 1))
                    nc.vector.tensor_copy(t_dots[0:1, c0 : c0 + cw],
                                          p_dot[0:1, 0:cw])
                nc.sync.dma_start(d_dots[b : b + 1, 0:sb], t_dots[0:1, 0:sb])

                # scatter into [N, M] grid prefilled with c_q
                t_dbuf = sml.tile([N, M], F32, tag="dbuf")
                t_cqc = sml.tile([N, 1], F32, tag="cqc")
                nc.sync.dma_start(
                    t_cqc,
                    bass.AP(tensor=d_f32b.tensor, offset=O_CQ + qi,
                            ap=[[0, N], [1, 1]]))
                nc.vector.tensor_scalar(t_dbuf, smaskf[:, b, :], 0.0, t_cqc,
                                        op0=ALU.mult, op1=ALU.add)
                for (L, r0, cnt, soff) in segs[b]:
                    nc.sync.dma_start(
                        t_dbuf[r0 : r0 + cnt, 0:L],
                        d_dots[b, soff : soff + cnt * L].rearrange(
                            "(c l) -> c l", l=L))

                # group softmax + probs scale + global softmax
                t_gmx = sml.tile([N, 1], F32, tag="gmx")
                nc.vector.tensor_reduce(t_gmx, t_dbuf, axis=AX.X, op=ALU.max)
                t_gnmx = sml.tile([N, 1], F32, tag="gnmx")
                nc.vector.tensor_scalar_mul(t_gnmx, t_gmx, -1.0)
                t_ex = sml.tile([N, M], F32, tag="ex")
                t_rs = sml.tile([N, 1], F32, tag="rs")
                nc.scalar.activation(t_ex, t_dbuf, AF.Exp, bias=t_gnmx,
                                     scale=1.0, accum_out=t_rs)
                t_ri = sml.tile([N, 1], F32, tag="ri")
                nc.vector.reciprocal(t_ri, t_rs)
                t_lg = sml.tile([N, M], F32, tag="lg")
                nc.vector.tensor_scalar(t_lg, t_ex, t_ri, s_prn[:, b : b + 1],
                                        op0=ALU.mult, op1=ALU.mult)
                t_gl = sml.tile([N, M], F32, tag="gl")
                t_grs = sml.tile([N, 1], F32, tag="grs")
                nc.scalar.activation(t_gl, t_lg, AF.Exp, accum_out=t_grs)
                t_gs = sml.tile([1, 1], F32, tag="gs")
                nc.gpsimd.tensor_reduce(t_gs, t_grs, axis=AX.C, op=ALU.add)
                t_gi = sml.tile([1, 1], F32, tag="gi")
                nc.vector.reciprocal(t_gi, t_gs)
                nc.sync.dma_start(d_ginv[b : b + 1, :], t_gi)
                t_gic = sml.tile([N, 1], F32, tag="gic")
                nc.sync.dma_start(
                    t_gic,
                    bass.AP(tensor=d_ginv.tensor, offset=b, ap=[[0, N], [1, 1]]))
                t_wa = sml.tile([N, M], BF, tag="wa")
                nc.vector.scalar_tensor_tensor(
                    t_wa, t_gl, t_gic, smaskf[:, b, :],
                    op0=ALU.mult, op1=ALU.mult)

                # gather wa back to compacted order
                for (L, r0, cnt, soff) in segs[b]:
                    nc.sync.dma_start(
                        d_wa[b, soff : soff + cnt * L].rearrange(
                            "(c l) -> c l", l=L),
                        t_wa[r0 : r0 + cnt, 0:L])
                t_wac = big.tile([1, S_MAX], BF, tag="wac")
                nc.sync.dma_start(t_wac[0:1, 0:sb], d_wa[b : b + 1, 0:sb])

                # pass 2: me[e] = sum_s nodesT[e, s] * wa[s]
                for et in range(ET):
                    for (c0, cw) in chs:
                        p_w = psw.tile([128, CHUNK], F32, tag="wb")
                        nc.tensor.matmul(p_w[:, 0:cw], s_onesb,
                                         t_wac[0:1, c0 : c0 + cw],
                                         start=True, stop=True)
                        t_wb = strm.tile([128, CHUNK], BF, tag="wb16")
                        nc.vector.tensor_copy(t_wb[:, 0:cw], p_w[:, 0:cw])
                        t_me = strm.tile([128, 1], F32, tag="me")
                        t_junk = strm.tile([128, CHUNK], BF, tag="junk")
                        nc.vector.scalar_tensor_tensor(
                            out=t_junk[:, 0:cw],
                            in0=t_nt[:, et, c0 : c0 + cw],
                            scalar=1.0,
                            in1=t_wb[:, 0:cw],
                            op0=ALU.mult, op1=ALU.mult,
                            accum_out=t_me)
                        partials[(qi, et)].append(t_me)
                if k == K - 1:
                    for et in range(ET):
                        ps_list = partials.pop((qi, et))
                        acc = ps_list[0]
                        for i, t in enumerate(ps_list[1:]):
                            if i == len(ps_list) - 2:
                                dst = t_pooled[:, et, qi : qi + 1]
                            else:
                                dst = strm.tile([128, 1], F32, tag="acc")
                            nc.vector.tensor_tensor(dst, acc, t, op=ALU.add)
                            acc = dst
            nc.sync.dma_start(d_pooled, t_pooled)

    nc.compile()
    return nc


_CACHE = {}


def kernel(**inputs) -> np.ndarray:
    per_core, prof2, S_b, segs, meta = _host_prep(inputs)
    key = prof2.tobytes()
    if key not in _CACHE:
        _CACHE[key] = _build_program(S_b, segs)
    nc = _CACHE[key]
    res = run_bass_kernel_spmd(nc, per_core, list(range(NCORES)))
    pooled = np.empty((Q, E), np.float32)
    for c in range(NCORES):
        pl = res.results[c]["pooled"]            # [128, ET, QPC]
        pooled[c * QPC : (c + 1) * QPC] = pl.transpose(1, 0, 2).reshape(E, QPC).T
    rows = np.tanh(pooled @ meta["Wg"] + meta["bg"])
    out = meta["hs"].copy()
    np.add.at(out, meta["gnn_idx"], rows)
    return out


# revision 3
# speedup vs baseline: 1.3728x; 1.3728x over previous
"""Trainium2 Bass kernel for the CustomGNNLayer problem (v4).

Wall-clock-oriented redesign. The graded metric is dominated by host prep +
axon transfer + per-call dispatch, so v2:
  - moves the tiny query-side math to host numpy (rel softmax via Wc, probs
    gather, gq = tanh(hs@Wq+b), c_q, and the final tanh(pooled@Wg+b) —
    ~0.2 GFLOP total), so Wc/Wq/Wg/onehot are never shipped;
  - ships compacted nodes as bf16 (halves bytes; PE matmuls run bf16);
  - device computes only the heavy per-slot part: X = tanh(Wn^T n + bn),
    dots = gq.X, group+global softmax, weighted sum -> pooled [E, QPC].

Device layout per core (one SPMD program, shapes uniform across cores):
  blocks b = qi*K + k; groups sorted by padded length desc; per-block class
  profile = position-wise max across cores of sorted padded lengths.
"""
import sys

sys.path.insert(0, "/opt/trn_rl_repo")

import numpy as np
from ml_dtypes import bfloat16, float8_e4m3

try:
    import jax
    jax.config.update("jax_compilation_cache_dir", "/tmp/jax_bass_cache")
    jax.config.update("jax_persistent_cache_min_compile_time_secs", 0.0)
    jax.config.update("jax_persistent_cache_min_entry_size_bytes", 0)
except Exception:
    pass

import concourse.bacc as bacc
import concourse.bass as bass
import concourse.tile as tile
from concourse import mybir
from concourse.bass_utils import run_bass_kernel_spmd

F32 = mybir.dt.float32

# f32 -> e4m3 via hardware f16 cast + 64K LUT (ml_dtypes' direct cast is
# scalar-slow; double-rounding differs by at most 1 ulp on ~0.4% of values,
# far inside this problem's accuracy budget).
import warnings
with warnings.catch_warnings():
    warnings.simplefilter("ignore")
    _E4M3_LUT = (np.arange(65536, dtype=np.uint16).view(np.float16)
                 .astype(float8_e4m3).view(np.uint8))


def _cast_e4m3_T(arr):
    """[S, E] f32 -> [E, S] e4m3 (transposed), fast path."""
    h = arr.T.astype(np.float16).view(np.uint16)
    return _E4M3_LUT.take(h).view(float8_e4m3)
BF = mybir.dt.bfloat16
FP8 = mybir.dt.float8e4
AF = mybir.ActivationFunctionType
ALU = mybir.AluOpType
AX = mybir.AxisListType

Q, K, N, M = 64, 2, 32, 64
E, D, R, S = 256, 1024, 200, 8192
NCORES = 8
QPC = Q // NCORES          # 8 queries per core
NB = QPC * K               # 16 blocks per core, b = qi*K + k
PAD = 4
CHUNK = 512
ET = E // 128              # 2 e-tiles
DT = D // 128              # 8 d-tiles
INV = 1.0 / (N * M * K)


def _chunks(s):
    n = (s + CHUNK - 1) // CHUNK
    h = s // 2
    base, rem = divmod(h, n)
    sizes = [2 * (base + (1 if i < rem else 0)) for i in range(n)]
    out, off = [], 0
    for sz in sizes:
        out.append((off, sz))
        off += sz
    return out


def _host_prep(inputs):
    hs = np.ascontiguousarray(inputs["hidden_states"], dtype=np.float32)
    nodes = np.asarray(inputs["nodes"], dtype=np.float32)
    prob_idx = np.asarray(inputs["prob_idx"])
    gnn_idx = np.asarray(inputs["gnn_idx"]).astype(np.int64)
    rel_idx = np.asarray(inputs["rel_idx"]).astype(np.int64)
    Wc = np.asarray(inputs["Wc"], np.float32)
    bc = np.asarray(inputs["bc"], np.float32)
    Wq = np.asarray(inputs["Wq"], np.float32)
    bq = np.asarray(inputs["bq"], np.float32)
    Wn = np.asarray(inputs["Wn"], np.float32)
    bn = np.asarray(inputs["bn"], np.float32)

    # ---- query-side math on host (tiny) ----
    rl = hs[rel_idx] @ Wc + bc                       # [Q, R]
    rl -= rl.max(axis=1, keepdims=True)
    np.exp(rl, out=rl)
    rl /= rl.sum(axis=1, keepdims=True)
    probs10 = np.take_along_axis(
        rl, prob_idx.reshape(Q, K * N), axis=1).reshape(Q, K, N) * 10.0
    gq = np.tanh(hs[gnn_idx] @ Wq + bq)              # [Q, D]
    c_q = gq @ np.tanh(bn)                           # [Q]

    # ---- compaction profile (kept slots are the prefix of each group) ----
    mask0 = nodes[..., 0] != 0.0                     # [Q,K,N,M]
    lens = mask0.sum(axis=3, dtype=np.int64)         # [Q,K,N]
    Lg = np.minimum(((np.maximum(lens, 1) + PAD - 1) // PAD) * PAD, M)
    Lg5 = Lg.reshape(NCORES, QPC, K, N)
    order = np.argsort(-Lg5, axis=3, kind="stable")  # [C,QPC,K,N]
    Lsort = -np.sort(-Lg5, axis=3)
    prof2 = Lsort.max(axis=0).reshape(NB, N)         # [NB, N] desc, same all cores
    S_b = [int(s) for s in prof2.sum(axis=1)]
    starts = np.concatenate(
        [np.zeros((NB, 1), np.int64), np.cumsum(prof2, axis=1)], axis=1)

    segs = []       # [NB] list of (L, row0, cnt, slot_off)
    for b in range(NB):
        p = prof2[b]
        s, off, i = [], 0, 0
        while i < N:
            j = i
            while j < N and p[j] == p[i]:
                j += 1
            L = int(p[i])
            s.append((L, i, j - i, off))
            off += L * (j - i)
            i = j
        segs.append(s)

    # position -> (rank, offset) maps per block
    pos_rank, pos_off = [], []
    for b in range(NB):
        pr = np.repeat(np.arange(N), prof2[b])
        pos_rank.append(pr)
        pos_off.append(np.arange(S_b[b]) - np.repeat(starts[b, :N], prof2[b]))

    # per-core small tensors
    lens5 = lens.reshape(NCORES, QPC, K, N)
    lens_ord = np.take_along_axis(lens5, order, axis=3)       # [C,QPC,K,N]
    maskf = ((np.arange(M)[None, None, None, None, :] <
              lens_ord[..., None]).astype(np.float32) * INV)  # [C,QPC,K,N,M]
    maskf = np.ascontiguousarray(maskf.reshape(NCORES, NB, N, M))
    pr_ord = np.take_along_axis(
        probs10.reshape(NCORES, QPC, K, N), order, axis=3).reshape(NCORES, NB, N)
    prn = np.ascontiguousarray(pr_ord.transpose(0, 2, 1), np.float32)  # [C,N,NB]

    Wn_bf = Wn.astype(bfloat16)
    S_tot = sum(S_b)

    def _core_stream(c):
        qs = np.arange(c * QPC, (c + 1) * QPC)
        buf = np.empty(E * S_tot, float8_e4m3)
        off = 0
        for qi in range(QPC):
            for k in range(K):
                b = qi * K + k
                sb = S_b[b]
                idx = order[c, qi, k][pos_rank[b]] * M + pos_off[b]
                arr = nodes[qs[qi], k].reshape(N * M, E)[idx]   # [sb, E] f32
                buf[off : off + E * sb] = _cast_e4m3_T(arr).ravel()
                off += E * sb
        return buf

    from concurrent.futures import ThreadPoolExecutor
    with ThreadPoolExecutor(NCORES) as tpe:
        bufs = list(tpe.map(_core_stream, range(NCORES)))

    per_core = []
    for c in range(NCORES):
        qs = np.arange(c * QPC, (c + 1) * QPC)
        buf = bufs[c]
        f32b = np.concatenate([
            np.ascontiguousarray(bn.reshape(DT, 128).T).ravel(),     # [128,DT]
            maskf[c].ravel(),                                        # [NB,N,M]
            prn[c].ravel(),                                          # [N,NB]
            c_q[qs].astype(np.float32).ravel(),                      # [QPC]
        ])
        bf16b = np.concatenate([
            np.ascontiguousarray(gq[qs].T).astype(bfloat16).ravel(), # [D,QPC]
            Wn_bf.ravel(),                                           # [E,D]
            np.ones(128, bfloat16),
        ])
        per_core.append({"nodesT": buf, "f32b": f32b, "bf16b": bf16b})
    meta = {
        "gnn_idx": gnn_idx, "hs": hs,
        "Wg": np.asarray(inputs["Wg"], np.float32),
        "bg": np.asarray(inputs["bg"], np.float32),
    }
    return per_core, prof2, S_b, segs, meta


def _build_program(S_b, segs):
    nc = bacc.Bacc("TRN2", target_bir_lowering=False, debug=False,
                   num_devices=NCORES)
    S_MAX = max(S_b)
    S_tot = sum(S_b)

    O_BN, O_MF = 0, 128 * DT
    O_PRN = O_MF + NB * N * M
    O_CQ = O_PRN + N * NB
    NF32 = O_CQ + QPC
    O_WN = D * QPC
    O_ONE = O_WN + E * D
    NBF = O_ONE + 128
    d_nodesT = nc.dram_tensor("nodesT", [E * S_tot], FP8, kind="ExternalInput").ap()
    d_f32b = nc.dram_tensor("f32b", [NF32], F32, kind="ExternalInput").ap()
    d_bf16b = nc.dram_tensor("bf16b", [NBF], BF, kind="ExternalInput").ap()
    d_pooled = nc.dram_tensor("pooled", [128, ET, QPC], F32,
                              kind="ExternalOutput").ap()

    # DRAM scratch
    d_dots = nc.dram_tensor("sc_dots", [NB, N * M], F32).ap()
    d_wa = nc.dram_tensor("sc_wa", [NB, N * M], BF).ap()
    d_ginv = nc.dram_tensor("sc_ginv", [NB, 1], F32).ap()

    with tile.TileContext(nc) as tc:
        with tc.tile_pool(name="wts", bufs=1) as wts, \
             tc.tile_pool(name="big", bufs=2) as big, \
             tc.tile_pool(name="strm", bufs=4) as strm, \
             tc.tile_pool(name="sml", bufs=4) as sml, \
             tc.tile_pool(name="ps", bufs=2, space="PSUM") as ps, \
             tc.tile_pool(name="psd", bufs=2, space="PSUM") as psd, \
             tc.tile_pool(name="psw", bufs=2, space="PSUM") as psw:

            # ---------------- load constants ----------------
            sWn = wts.tile([128, ET, D], BF)
            nc.sync.dma_start(sWn, bass.AP(tensor=d_bf16b.tensor, offset=O_WN,
                                           ap=[[D, 128], [128 * D, ET], [1, D]]))
            t_gqT = wts.tile([128, DT, QPC], BF)
            nc.sync.dma_start(t_gqT, bass.AP(tensor=d_bf16b.tensor, offset=0,
                                             ap=[[QPC, 128], [128 * QPC, DT],
                                                 [1, QPC]]))
            sbn = wts.tile([128, DT], F32)
            nc.sync.dma_start(sbn, bass.AP(tensor=d_f32b.tensor, offset=O_BN,
                                           ap=[[DT, 128], [1, DT]]))
            smaskf = wts.tile([N, NB, M], F32)
            nc.sync.dma_start(smaskf, bass.AP(tensor=d_f32b.tensor, offset=O_MF,
                                              ap=[[M, N], [N * M, NB], [1, M]]))
            s_prn = wts.tile([N, NB], F32)
            nc.sync.dma_start(s_prn, bass.AP(tensor=d_f32b.tensor, offset=O_PRN,
                                             ap=[[NB, N], [1, NB]]))
            s_onesb = wts.tile([1, 128], BF)
            nc.sync.dma_start(s_onesb, bass.AP(tensor=d_bf16b.tensor, offset=O_ONE,
                                               ap=[[128, 1], [1, 128]]))
            t_pooled = wts.tile([128, ET, QPC], F32)

            # ---------------- main loop ----------------
            nt_off = 0
            from collections import defaultdict
            partials = defaultdict(list)
            for b in range(NB):
                qi, k = b // K, b % K
                sb = S_b[b]
                chs = _chunks(sb)

                t_nt8 = big.tile([128, ET, S_MAX], FP8, tag="nt8")
                nc.sync.dma_start(
                    t_nt8[:, :, 0:sb],
                    bass.AP(tensor=d_nodesT.tensor, offset=nt_off,
                            ap=[[sb, 128], [128 * sb, ET], [1, sb]]))
                nt_off += E * sb
                t_nt = big.tile([128, ET, S_MAX], BF, tag="nt")
                for et in range(ET):
                    nc.vector.tensor_copy(t_nt[:, et, 0:sb], t_nt8[:, et, 0:sb])

                # dots over the compacted stream
                t_dots = big.tile([1, S_MAX], F32, tag="dots")
                for (c0, cw) in chs:
                    p_dot = psd.tile([1, CHUNK], F32, tag="dot")
                    for dt_i in range(DT):
                        p_x = ps.tile([128, CHUNK], F32, tag="mm")
                        for et in range(ET):
                            nc.tensor.matmul(
                                p_x[:, 0:cw],
                                sWn[:, et, dt_i * 128:(dt_i + 1) * 128],
                                t_nt[:, et, c0 : c0 + cw],
                                start=(et == 0), stop=(et == ET - 1))
                        t_tx = strm.tile([128, CHUNK], BF, tag="tx")
                        nc.scalar.activation(t_tx[:, 0:cw], p_x[:, 0:cw],
                                             AF.Tanh, bias=sbn[:, dt_i : dt_i + 1],
                                             scale=1.0)
                        nc.tensor.matmul(p_dot[0:1, 0:cw],
                                         t_gqT[:, dt_i, qi : qi + 1],
                                         t_tx[:, 0:cw], start=(dt_i == 0),
                                         stop=(dt_i == DT - 1))
                    nc.vector.tensor_copy(t_dots[0:1, c0 : c0 + cw],
                                          p_dot[0:1, 0:cw])
                nc.sync.dma_start(d_dots[b : b + 1, 0:sb], t_dots[0:1, 0:sb])

                # scatter into [N, M] grid prefilled with c_q
                t_dbuf = sml.tile([N, M], F32, tag="dbuf")
                t_cqc = sml.tile([N, 1], F32, tag="cqc")
                nc.sync.dma_start(
                    t_cqc,
                    bass.AP(tensor=d_f32b.tensor, offset=O_CQ + qi,
                            ap=[[0, N], [1, 1]]))
                nc.vector.tensor_scalar(t_dbuf, smaskf[:, b, :], 0.0, t_cqc,
                                        op0=ALU.mult, op1=ALU.add)
                for (L, r0, cnt, soff) in segs[b]:
                    nc.sync.dma_start(
                        t_dbuf[r0 : r0 + cnt, 0:L],
                        d_dots[b, soff : soff + cnt * L].rearrange(
                            "(c l) -> c l", l=L))

                # group softmax + probs scale + global softmax
                t_gmx = sml.tile([N, 1], F32, tag="gmx")
                nc.vector.tensor_reduce(t_gmx, t_dbuf, axis=AX.X, op=ALU.max)
                t_gnmx = sml.tile([N, 1], F32, tag="gnmx")
                nc.vector.tensor_scalar_mul(t_gnmx, t_gmx, -1.0)
                t_ex = sml.tile([N, M], F32, tag="ex")
                t_rs = sml.tile([N, 1], F32, tag="rs")
                nc.scalar.activation(t_ex, t_dbuf, AF.Exp, bias=t_gnmx,
                                     scale=1.0, accum_out=t_rs)
                t_ri = sml.tile([N, 1], F32, tag="ri")
                nc.vector.reciprocal(t_ri, t_rs)
                t_lg = sml.tile([N, M], F32, tag="lg")
                nc.vector.tensor_scalar(t_lg, t_ex, t_ri, s_prn[:, b : b + 1],
                                        op0=ALU.mult, op1=ALU.mult)
                t_gl = sml.tile([N, M], F32, tag="gl")
                t_grs = sml.tile([N, 1], F32, tag="grs")
                nc.scalar.activation(t_gl, t_lg, AF.Exp, accum_out=t_grs)
                t_gs = sml.tile([1, 1], F32, tag="gs")
                nc.gpsimd.tensor_reduce(t_gs, t_grs, axis=AX.C, op=ALU.add)
                t_gi = sml.tile([1, 1], F32, tag="gi")
                nc.vector.reciprocal(t_gi, t_gs)
                nc.sync.dma_start(d_ginv[b : b + 1, :], t_gi)
                t_gic = sml.tile([N, 1], F32, tag="gic")
                nc.sync.dma_start(
                    t_gic,
                    bass.AP(tensor=d_ginv.tensor, offset=b, ap=[[0, N], [1, 1]]))
                t_wa = sml.tile([N, M], BF, tag="wa")
                nc.vector.scalar_tensor_tensor(
                    t_wa, t_gl, t_gic, smaskf[:, b, :],
                    op0=ALU.mult, op1=ALU.mult)

                # gather wa back to compacted order
                for (L, r0, cnt, soff) in segs[b]:
                    nc.sync.dma_start(
                        d_wa[b, soff : soff + cnt * L].rearrange(
                            "(c l) -> c l", l=L),
                        t_wa[r0 : r0 + cnt, 0:L])
                t_wac = big.tile([1, S_MAX], BF, tag="wac")
                nc.sync.dma_start(t_wac[0:1, 0:sb], d_wa[b : b + 1, 0:sb])

                # pass 2: me[e] = sum_s nodesT[e, s] * wa[s]
                for et in range(ET):
                    for (c0, cw) in chs:
                        p_w = psw.tile([128, CHUNK], F32, tag="wb")
                        nc.tensor.matmul(p_w[:, 0:cw], s_onesb,
                                         t_wac[0:1, c0 : c0 + cw],
                                         start=True, stop=True)
                        t_wb = strm.tile([128, CHUNK], BF, tag="wb16")
                        nc.vector.tensor_copy(t_wb[:, 0:cw], p_w[:, 0:cw])
                        t_me = strm.tile([128, 1], F32, tag="me")
                        t_junk = strm.tile([128, CHUNK], BF, tag="junk")
                        nc.vector.scalar_tensor_tensor(
                            out=t_junk[:, 0:cw],
                            in0=t_nt[:, et, c0 : c0 + cw],
                            scalar=1.0,
                            in1=t_wb[:, 0:cw],
                            op0=ALU.mult, op1=ALU.mult,
                            accum_out=t_me)
                        partials[(qi, et)].append(t_me)
                if k == K - 1:
                    for et in range(ET):
                        ps_list = partials.pop((qi, et))
                        acc = ps_list[0]
                        for i, t in enumerate(ps_list[1:]):
                            if i == len(ps_list) - 2:
                                dst = t_pooled[:, et, qi : qi + 1]
                            else:
                                dst = strm.tile([128, 1], F32, tag="acc")
                            nc.vector.tensor_tensor(dst, acc, t, op=ALU.add)
                            acc = dst
            nc.sync.dma_start(d_pooled, t_pooled)

    nc.compile()
    return nc


_CACHE = {}


def kernel(**inputs) -> np.ndarray:
    per_core, prof2, S_b, segs, meta = _host_prep(inputs)
    key = prof2.tobytes()
    if key not in _CACHE:
        _CACHE[key] = _build_program(S_b, segs)
    nc = _CACHE[key]
    res = run_bass_kernel_spmd(nc, per_core, list(range(NCORES)))
    pooled = np.empty((Q, E), np.float32)
    for c in range(NCORES):
        pl = res.results[c]["pooled"]            # [128, ET, QPC]
        pooled[c * QPC : (c + 1) * QPC] = pl.transpose(1, 0, 2).reshape(E, QPC).T
    rows = np.tanh(pooled @ meta["Wg"] + meta["bg"])
    out = meta["hs"].copy()
    np.add.at(out, meta["gnn_idx"], rows)
    return out


# revision 4
# speedup vs baseline: 1.9492x; 1.4199x over previous
"""Trainium2 Bass kernel for the CustomGNNLayer problem (v4).

Wall-clock-oriented redesign. The graded metric is dominated by host prep +
axon transfer + per-call dispatch, so v2:
  - moves the tiny query-side math to host numpy (rel softmax via Wc, probs
    gather, gq = tanh(hs@Wq+b), c_q, and the final tanh(pooled@Wg+b) —
    ~0.2 GFLOP total), so Wc/Wq/Wg/onehot are never shipped;
  - ships compacted nodes as bf16 (halves bytes; PE matmuls run bf16);
  - device computes only the heavy per-slot part: X = tanh(Wn^T n + bn),
    dots = gq.X, group+global softmax, weighted sum -> pooled [E, QPC].

Device layout per core (one SPMD program, shapes uniform across cores):
  blocks b = qi*K + k; groups sorted by padded length desc; per-block class
  profile = position-wise max across cores of sorted padded lengths.
"""
import sys

sys.path.insert(0, "/opt/trn_rl_repo")

import numpy as np
from ml_dtypes import bfloat16, float8_e4m3

try:
    import jax
    jax.config.update("jax_compilation_cache_dir", "/tmp/jax_bass_cache")
    jax.config.update("jax_persistent_cache_min_compile_time_secs", 0.0)
    jax.config.update("jax_persistent_cache_min_entry_size_bytes", 0)
except Exception:
    pass

import concourse.bacc as bacc
import concourse.bass as bass
import concourse.tile as tile
from concourse import mybir
from concourse.bass_utils import run_bass_kernel_spmd

F32 = mybir.dt.float32

# f32 -> e4m3 via hardware f16 cast + 64K LUT (ml_dtypes' direct cast is
# scalar-slow; double-rounding differs by at most 1 ulp on ~0.4% of values,
# far inside this problem's accuracy budget).
import warnings
with warnings.catch_warnings():
    warnings.simplefilter("ignore")
    _E4M3_LUT = (np.arange(65536, dtype=np.uint16).view(np.float16)
                 .astype(float8_e4m3).view(np.uint8))


def _cast_e4m3_T(arr):
    """[S, E] f32 -> [E, S] e4m3 (transposed), fast path."""
    h = arr.T.astype(np.float16).view(np.uint16)
    return _E4M3_LUT.take(h).view(float8_e4m3)
BF = mybir.dt.bfloat16
FP8 = mybir.dt.float8e4
AF = mybir.ActivationFunctionType
ALU = mybir.AluOpType
AX = mybir.AxisListType

Q, K, N, M = 64, 2, 32, 64
E, D, R, S = 256, 1024, 200, 8192
NCORES = 8
QPC = Q // NCORES          # 8 queries per core
NB = QPC * K               # 16 blocks per core, b = qi*K + k
PAD = 4
CHUNK = 512
ET = E // 128              # 2 e-tiles
DT = D // 128              # 8 d-tiles
INV = 1.0 / (N * M * K)


def _chunks(s):
    n = (s + CHUNK - 1) // CHUNK
    h = s // 2
    base, rem = divmod(h, n)
    sizes = [2 * (base + (1 if i < rem else 0)) for i in range(n)]
    out, off = [], 0
    for sz in sizes:
        out.append((off, sz))
        off += sz
    return out


def _host_prep(inputs):
    hs = np.ascontiguousarray(inputs["hidden_states"], dtype=np.float32)
    nodes = np.asarray(inputs["nodes"], dtype=np.float32)
    prob_idx = np.asarray(inputs["prob_idx"])
    gnn_idx = np.asarray(inputs["gnn_idx"]).astype(np.int64)
    rel_idx = np.asarray(inputs["rel_idx"]).astype(np.int64)
    Wc = np.asarray(inputs["Wc"], np.float32)
    bc = np.asarray(inputs["bc"], np.float32)
    Wq = np.asarray(inputs["Wq"], np.float32)
    bq = np.asarray(inputs["bq"], np.float32)
    Wn = np.asarray(inputs["Wn"], np.float32)
    bn = np.asarray(inputs["bn"], np.float32)

    # ---- query-side math on host (tiny) ----
    rl = hs[rel_idx] @ Wc + bc                       # [Q, R]
    rl -= rl.max(axis=1, keepdims=True)
    np.exp(rl, out=rl)
    rl /= rl.sum(axis=1, keepdims=True)
    probs10 = np.take_along_axis(
        rl, prob_idx.reshape(Q, K * N), axis=1).reshape(Q, K, N) * 10.0
    gq = np.tanh(hs[gnn_idx] @ Wq + bq)              # [Q, D]
    c_q = gq @ np.tanh(bn)                           # [Q]

    # ---- compaction profile (kept slots are the prefix of each group) ----
    mask0 = nodes[..., 0] != 0.0                     # [Q,K,N,M]
    lens = mask0.sum(axis=3, dtype=np.int64)         # [Q,K,N]
    Lg = np.minimum(((np.maximum(lens, 1) + PAD - 1) // PAD) * PAD, M)
    Lg5 = Lg.reshape(NCORES, QPC, K, N)
    order = np.argsort(-Lg5, axis=3, kind="stable")  # [C,QPC,K,N]
    Lsort = -np.sort(-Lg5, axis=3)
    prof2 = Lsort.max(axis=0).reshape(NB, N)         # [NB, N] desc, same all cores
    S_b = [int(s) for s in prof2.sum(axis=1)]
    starts = np.concatenate(
        [np.zeros((NB, 1), np.int64), np.cumsum(prof2, axis=1)], axis=1)

    pkey = prof2.tobytes()
    cached = _PREP_CACHE.get(pkey)
    if cached is None:
        segs = []       # [NB] list of (L, row0, cnt, slot_off)
        for b in range(NB):
            p = prof2[b]
            s, off, i = [], 0, 0
            while i < N:
                j = i
                while j < N and p[j] == p[i]:
                    j += 1
                L = int(p[i])
                s.append((L, i, j - i, off))
                off += L * (j - i)
                i = j
            segs.append(s)
        # position -> (rank, offset) maps per block
        pos_rank, pos_off = [], []
        for b in range(NB):
            pos_rank.append(np.repeat(np.arange(N), prof2[b]))
            pos_off.append(
                np.arange(S_b[b]) - np.repeat(starts[b, :N], prof2[b]))
        _PREP_CACHE[pkey] = (segs, pos_rank, pos_off)
    else:
        segs, pos_rank, pos_off = cached

    # per-core small tensors
    lens5 = lens.reshape(NCORES, QPC, K, N)
    lens_ord = np.take_along_axis(lens5, order, axis=3)       # [C,QPC,K,N]
    maskf = ((np.arange(M)[None, None, None, None, :] <
              lens_ord[..., None]).astype(np.float32) * INV)  # [C,QPC,K,N,M]
    maskf = np.ascontiguousarray(maskf.reshape(NCORES, NB, N, M))
    pr_ord = np.take_along_axis(
        probs10.reshape(NCORES, QPC, K, N), order, axis=3).reshape(NCORES, NB, N)
    prn = np.ascontiguousarray(pr_ord.transpose(0, 2, 1), np.float32)  # [C,N,NB]

    Wn_bf = Wn.astype(bfloat16)
    S_tot = sum(S_b)

    def _core_stream(c):
        qs = np.arange(c * QPC, (c + 1) * QPC)
        buf = np.empty(E * S_tot, float8_e4m3)
        off = 0
        for qi in range(QPC):
            for k in range(K):
                b = qi * K + k
                sb = S_b[b]
                idx = order[c, qi, k][pos_rank[b]] * M + pos_off[b]
                arr = nodes[qs[qi], k].reshape(N * M, E)[idx]   # [sb, E] f32
                buf[off : off + E * sb] = _cast_e4m3_T(arr).ravel()
                off += E * sb
        return buf

    from concurrent.futures import ThreadPoolExecutor
    with ThreadPoolExecutor(NCORES) as tpe:
        bufs = list(tpe.map(_core_stream, range(NCORES)))

    per_core = []
    for c in range(NCORES):
        qs = np.arange(c * QPC, (c + 1) * QPC)
        buf = bufs[c]
        f32b = np.concatenate([
            np.ascontiguousarray(bn.reshape(DT, 128).T).ravel(),     # [128,DT]
            maskf[c].ravel(),                                        # [NB,N,M]
            prn[c].ravel(),                                          # [N,NB]
            c_q[qs].astype(np.float32).ravel(),                      # [QPC]
        ])
        bf16b = np.concatenate([
            np.ascontiguousarray(gq[qs].T).astype(bfloat16).ravel(), # [D,QPC]
            Wn_bf.ravel(),                                           # [E,D]
            np.ones(128, bfloat16),
        ])
        per_core.append({"nodesT": buf, "f32b": f32b, "bf16b": bf16b})
    meta = {
        "gnn_idx": gnn_idx, "hs": hs,
        "Wg": np.asarray(inputs["Wg"], np.float32),
        "bg": np.asarray(inputs["bg"], np.float32),
    }
    return per_core, prof2, S_b, segs, meta


def _build_program(S_b, segs):
    nc = bacc.Bacc("TRN2", target_bir_lowering=False, debug=False,
                   num_devices=NCORES)
    S_MAX = max(S_b)
    S_tot = sum(S_b)

    O_BN, O_MF = 0, 128 * DT
    O_PRN = O_MF + NB * N * M
    O_CQ = O_PRN + N * NB
    NF32 = O_CQ + QPC
    O_WN = D * QPC
    O_ONE = O_WN + E * D
    NBF = O_ONE + 128
    d_nodesT = nc.dram_tensor("nodesT", [E * S_tot], FP8, kind="ExternalInput").ap()
    d_f32b = nc.dram_tensor("f32b", [NF32], F32, kind="ExternalInput").ap()
    d_bf16b = nc.dram_tensor("bf16b", [NBF], BF, kind="ExternalInput").ap()
    d_pooled = nc.dram_tensor("pooled", [128, ET, QPC], F32,
                              kind="ExternalOutput").ap()

    # DRAM scratch
    d_dots = nc.dram_tensor("sc_dots", [NB, N * M], F32).ap()
    d_wa = nc.dram_tensor("sc_wa", [NB, N * M], BF).ap()
    d_ginv = nc.dram_tensor("sc_ginv", [NB, 1], F32).ap()

    with tile.TileContext(nc) as tc:
        with tc.tile_pool(name="wts", bufs=1) as wts, \
             tc.tile_pool(name="big", bufs=2) as big, \
             tc.tile_pool(name="strm", bufs=4) as strm, \
             tc.tile_pool(name="sml", bufs=4) as sml, \
             tc.tile_pool(name="ps", bufs=2, space="PSUM") as ps, \
             tc.tile_pool(name="psd", bufs=2, space="PSUM") as psd, \
             tc.tile_pool(name="psw", bufs=2, space="PSUM") as psw:

            # ---------------- load constants ----------------
            sWn = wts.tile([128, ET, D], BF)
            nc.sync.dma_start(sWn, bass.AP(tensor=d_bf16b.tensor, offset=O_WN,
                                           ap=[[D, 128], [128 * D, ET], [1, D]]))
            t_gqT = wts.tile([128, DT, QPC], BF)
            nc.sync.dma_start(t_gqT, bass.AP(tensor=d_bf16b.tensor, offset=0,
                                             ap=[[QPC, 128], [128 * QPC, DT],
                                                 [1, QPC]]))
            sbn = wts.tile([128, DT], F32)
            nc.sync.dma_start(sbn, bass.AP(tensor=d_f32b.tensor, offset=O_BN,
                                           ap=[[DT, 128], [1, DT]]))
            smaskf = wts.tile([N, NB, M], F32)
            nc.sync.dma_start(smaskf, bass.AP(tensor=d_f32b.tensor, offset=O_MF,
                                              ap=[[M, N], [N * M, NB], [1, M]]))
            s_prn = wts.tile([N, NB], F32)
            nc.sync.dma_start(s_prn, bass.AP(tensor=d_f32b.tensor, offset=O_PRN,
                                             ap=[[NB, N], [1, NB]]))
            s_onesb = wts.tile([1, 128], BF)
            nc.sync.dma_start(s_onesb, bass.AP(tensor=d_bf16b.tensor, offset=O_ONE,
                                               ap=[[128, 1], [1, 128]]))
            t_pooled = wts.tile([128, ET, QPC], F32)

            # ---------------- main loop ----------------
            nt_off = 0
            from collections import defaultdict
            partials = defaultdict(list)
            for b in range(NB):
                qi, k = b // K, b % K
                sb = S_b[b]
                chs = _chunks(sb)

                t_nt8 = big.tile([128, ET, S_MAX], FP8, tag="nt8")
                nc.sync.dma_start(
                    t_nt8[:, :, 0:sb],
                    bass.AP(tensor=d_nodesT.tensor, offset=nt_off,
                            ap=[[sb, 128], [128 * sb, ET], [1, sb]]))
                nt_off += E * sb
                t_nt = big.tile([128, ET, S_MAX], BF, tag="nt")
                for et in range(ET):
                    nc.vector.tensor_copy(t_nt[:, et, 0:sb], t_nt8[:, et, 0:sb])

                # dots over the compacted stream
                t_dots = big.tile([1, S_MAX], F32, tag="dots")
                for (c0, cw) in chs:
                    p_dot = psd.tile([1, CHUNK], F32, tag="dot")
                    for dt_i in range(DT):
                        p_x = ps.tile([128, CHUNK], F32, tag="mm")
                        for et in range(ET):
                            nc.tensor.matmul(
                                p_x[:, 0:cw],
                                sWn[:, et, dt_i * 128:(dt_i + 1) * 128],
                                t_nt[:, et, c0 : c0 + cw],
                                start=(et == 0), stop=(et == ET - 1))
                        t_tx = strm.tile([128, CHUNK], BF, tag="tx")
                        nc.scalar.activation(t_tx[:, 0:cw], p_x[:, 0:cw],
                                             AF.Tanh, bias=sbn[:, dt_i : dt_i + 1],
                                             scale=1.0)
                        nc.tensor.matmul(p_dot[0:1, 0:cw],
                                         t_gqT[:, dt_i, qi : qi + 1],
                                         t_tx[:, 0:cw], start=(dt_i == 0),
                                         stop=(dt_i == DT - 1))
                    nc.vector.tensor_copy(t_dots[0:1, c0 : c0 + cw],
                                          p_dot[0:1, 0:cw])
                nc.sync.dma_start(d_dots[b : b + 1, 0:sb], t_dots[0:1, 0:sb])

                # scatter into [N, M] grid prefilled with c_q
                t_dbuf = sml.tile([N, M], F32, tag="dbuf")
                t_cqc = sml.tile([N, 1], F32, tag="cqc")
                nc.sync.dma_start(
                    t_cqc,
                    bass.AP(tensor=d_f32b.tensor, offset=O_CQ + qi,
                            ap=[[0, N], [1, 1]]))
                nc.vector.tensor_scalar(t_dbuf, smaskf[:, b, :], 0.0, t_cqc,
                                        op0=ALU.mult, op1=ALU.add)
                for (L, r0, cnt, soff) in segs[b]:
                    nc.sync.dma_start(
                        t_dbuf[r0 : r0 + cnt, 0:L],
                        d_dots[b, soff : soff + cnt * L].rearrange(
                            "(c l) -> c l", l=L))

                # group softmax + probs scale + global softmax
                t_gmx = sml.tile([N, 1], F32, tag="gmx")
                nc.vector.tensor_reduce(t_gmx, t_dbuf, axis=AX.X, op=ALU.max)
                t_gnmx = sml.tile([N, 1], F32, tag="gnmx")
                nc.vector.tensor_scalar_mul(t_gnmx, t_gmx, -1.0)
                t_ex = sml.tile([N, M], F32, tag="ex")
                t_rs = sml.tile([N, 1], F32, tag="rs")
                nc.scalar.activation(t_ex, t_dbuf, AF.Exp, bias=t_gnmx,
                                     scale=1.0, accum_out=t_rs)
                t_ri = sml.tile([N, 1], F32, tag="ri")
                nc.vector.reciprocal(t_ri, t_rs)
                t_lg = sml.tile([N, M], F32, tag="lg")
                nc.vector.tensor_scalar(t_lg, t_ex, t_ri, s_prn[:, b : b + 1],
                                        op0=ALU.mult, op1=ALU.mult)
                t_gl = sml.tile([N, M], F32, tag="gl")
                t_grs = sml.tile([N, 1], F32, tag="grs")
                nc.scalar.activation(t_gl, t_lg, AF.Exp, accum_out=t_grs)
                t_gs = sml.tile([1, 1], F32, tag="gs")
                nc.gpsimd.tensor_reduce(t_gs, t_grs, axis=AX.C, op=ALU.add)
                t_gi = sml.tile([1, 1], F32, tag="gi")
                nc.vector.reciprocal(t_gi, t_gs)
                nc.sync.dma_start(d_ginv[b : b + 1, :], t_gi)
                t_gic = sml.tile([N, 1], F32, tag="gic")
                nc.sync.dma_start(
                    t_gic,
                    bass.AP(tensor=d_ginv.tensor, offset=b, ap=[[0, N], [1, 1]]))
                t_wa = sml.tile([N, M], BF, tag="wa")
                nc.vector.scalar_tensor_tensor(
                    t_wa, t_gl, t_gic, smaskf[:, b, :],
                    op0=ALU.mult, op1=ALU.mult)

                # gather wa back to compacted order
                for (L, r0, cnt, soff) in segs[b]:
                    nc.sync.dma_start(
                        d_wa[b, soff : soff + cnt * L].rearrange(
                            "(c l) -> c l", l=L),
                        t_wa[r0 : r0 + cnt, 0:L])
                t_wac = big.tile([1, S_MAX], BF, tag="wac")
                nc.sync.dma_start(t_wac[0:1, 0:sb], d_wa[b : b + 1, 0:sb])

                # pass 2: me[e] = sum_s nodesT[e, s] * wa[s]
                for et in range(ET):
                    for (c0, cw) in chs:
                        p_w = psw.tile([128, CHUNK], F32, tag="wb")
                        nc.tensor.matmul(p_w[:, 0:cw], s_onesb,
                                         t_wac[0:1, c0 : c0 + cw],
                                         start=True, stop=True)
                        t_wb = strm.tile([128, CHUNK], BF, tag="wb16")
                        nc.vector.tensor_copy(t_wb[:, 0:cw], p_w[:, 0:cw])
                        t_me = strm.tile([128, 1], F32, tag="me")
                        t_junk = strm.tile([128, CHUNK], BF, tag="junk")
                        nc.vector.scalar_tensor_tensor(
                            out=t_junk[:, 0:cw],
                            in0=t_nt[:, et, c0 : c0 + cw],
                            scalar=1.0,
                            in1=t_wb[:, 0:cw],
                            op0=ALU.mult, op1=ALU.mult,
                            accum_out=t_me)
                        partials[(qi, et)].append(t_me)
                if k == K - 1:
                    for et in range(ET):
                        ps_list = partials.pop((qi, et))
                        acc = ps_list[0]
                        for i, t in enumerate(ps_list[1:]):
                            if i == len(ps_list) - 2:
                                dst = t_pooled[:, et, qi : qi + 1]
                            else:
                                dst = strm.tile([128, 1], F32, tag="acc")
                            nc.vector.tensor_tensor(dst, acc, t, op=ALU.add)
                            acc = dst
            nc.sync.dma_start(d_pooled, t_pooled)

    nc.compile()
    return nc


_CACHE = {}
_PREP_CACHE = {}


def kernel(**inputs) -> np.ndarray:
    per_core, prof2, S_b, segs, meta = _host_prep(inputs)
    key = prof2.tobytes()
    if key not in _CACHE:
        _CACHE[key] = _build_program(S_b, segs)
    nc = _CACHE[key]
    res = run_bass_kernel_spmd(nc, per_core, list(range(NCORES)))
    pooled = np.empty((Q, E), np.float32)
    for c in range(NCORES):
        pl = res.results[c]["pooled"]            # [128, ET, QPC]
        pooled[c * QPC : (c + 1) * QPC] = pl.transpose(1, 0, 2).reshape(E, QPC).T
    rows = np.tanh(pooled @ meta["Wg"] + meta["bg"])
    out = meta["hs"].copy()
    np.add.at(out, meta["gnn_idx"], rows)
    return out
